# revision 62
# baseline (speedup 1.0000x reference)
# Dot-product attention with per-batch key masking (d2l masked_softmax
# style), distributed over 8 NeuronCores:
#
#   out[b] = softmax(mask(Q[b] @ K[b]^T / sqrt(D), valid_lens[b])) @ V[b]
#
# Shapes: Q/K/V [32, 2048, 64] f32, valid_lens [32] i32.
#
# Strategy v2 (per core: 8 "jobs", each = (batch, query-half) with
# q-width 1024; the 64 (batch, half) jobs are sorted by valid_len and
# grouped 8-per-slot so slot s of every core shares a compile-time
# k-tile trip count kcs[s] — finer grouping than whole batches cuts the
# sum of per-slot maxima from 41 to ~38 k-tile equivalents):
#   - Host pre-transposes Q, K to d-major [D, S] fp16 (layout prep), so
#     mm1 runs with the d=64 contraction on partitions 0-63: scoresT
#     [k-tile 128, q 1024] = KT_tile.T @ QT in two N=512 matmuls into
#     one [128, 1024] PSUM tile (2 banks, tag ring of 2).
#   - attnT = Exp(scoresT/8 + mask_bias) on the ACT engine in ONE
#     [128, 1024] call per k-tile ((1024+172)/1.2GHz ~ 1.0us); ACT is
#     the bottleneck engine, kept back-to-back across job boundaries.
#   - mm2: oaug[d', q] += Vaug_tile.T @ attnT into a [65, 1024] PSUM
#     accumulator (2 banks), tag ring of 2 so job j+1 accumulates while
#     job j's tail drains (the v1 single accumulator serialized ~31us
#     of tail work after the last exp).
#   - Tail (split in two so the PE queue never heads-of-line blocks on
#     the DVE escape copy): (a) DVE PSUM->SBUF copy of oaug; (b) two
#     steps later: 8 PE transposes back to [q, d'] (into a tro tile
#     aliased into the oaug PSUM ring), reciprocal of the denominator
#     column, 8 per-q-block scalar multiplies, DMA out.
#   - A dummy exp on a zero constant issues at t=0 so the one-time ACT
#     table load (~1.3us) overlaps the initial DMAs.
#
# Engine-overlap schedule: software pipeline over (job, k-tile) steps
# with LAG=2 (mm2 of step i emitted after mm1+exp of steps i+1, i+2);
# job loads prefetched one job ahead, first k-tile's K chunk DMA'd
# separately so step-0 compute starts as soon as it lands.

import numpy as np
import ml_dtypes
from contextlib import ExitStack

import concourse.bass as bass
import concourse.bacc as bacc
import concourse.mybir as mybir
import concourse.tile as tile
from concourse.masks import make_identity
from concourse.bass_utils import run_bass_kernel_spmd

P = 128
S = 2048
D = 64
NT = S // P
NCORES = 8
NSLOTS = 8          # jobs per core
QW = 1024           # query width per job
NQB = QW // P       # q-blocks per job (8)
NHALF = S // QW     # query halves per batch (2)
MASK_NEG = -1.0e6
MASK_FP16 = -60000.0  # mask bias packed in fp16; exp underflows to 0 either way
SCALE = 0.125
F32 = mybir.dt.float32
F32R = mybir.dt.float32r
FP16 = mybir.dt.float16
BF16 = mybir.dt.bfloat16

MM1_FP16 = True
# The final jobs' tail multiplies split between DVE and the (then idle)
# ACT engine — contiguous halves so only one cross-engine sync boundary
# exists per job.
ACT_TAIL_JOBS = 2

LAST_RESULT = None
_PROGRAM_CACHE = {}


def _build_program(kcs, repeat=1, loop=1):
    import os
    abl = os.environ.get("ABL", "")  # perf-ablation probes: 'nomm2','notail'
    nc = bacc.Bacc("TRN2", target_bir_lowering=False, debug=False)

    qk_dt = FP16 if MM1_FP16 else F32R
    # qt and K packed in one DRAM tensor ([k-tile0 | qt | k-tiles 1..]) so
    # the latency-critical first load (k-tile 0 + qt) is a single DMA.
    # Rows 64-127 are host-zeroed so every matmul (mm1 AND mm2) contracts
    # over the full 128 partitions: uniform PE row configuration avoids
    # the per-switch array reconfiguration cost measured on hardware.
    kt_d = nc.dram_tensor(
        "kt", [NSLOTS, P, QW + S], qk_dt, kind="ExternalInput"
    )
    # V packed host-side as [V | 1 | mask] per k-tile row: the whole
    # (V, softmax-denominator ones column, exp mask bias) block loads as
    # ONE contiguous ~2KB descriptor per partition — the naive strided
    # [P, 0:kc, D] V load costs 128*kc tiny 128B descriptors, which on
    # real hardware is expensive enough to push DMA past the ACT roofline.
    v_d = nc.dram_tensor(
        "v", [NSLOTS, P, NT, D + 2], FP16, kind="ExternalInput"
    )
    o_d = nc.dram_tensor("out", [NSLOTS, P, NQB, D], F32, kind="ExternalOutput")

    with ExitStack() as ctx:
        tc = ctx.enter_context(tile.TileContext(nc))
        consts = ctx.enter_context(tc.tile_pool(name="consts", bufs=1))
        tp = ctx.enter_context(tc.tile_pool(name="tp", bufs=2))
        vp = ctx.enter_context(tc.tile_pool(name="vp", bufs=2))
        atp = ctx.enter_context(tc.tile_pool(name="atp", bufs=6))
        op_ = ctx.enter_context(tc.tile_pool(name="op_", bufs=2))
        sm = ctx.enter_context(tc.tile_pool(name="sm", bufs=2))
        ps = ctx.enter_context(tc.tile_pool(name="ps", bufs=2, space="PSUM"))

        # Dummy exp at t=0: triggers the one-time ACT table load with no
        # data deps so it overlaps the initial DMAs (DVE memset so the
        # dependency clears within ~0.1us).
        warm_in = consts.tile([P, 1], F32)
        nc.vector.memset(warm_in, 0.0)
        warm_out = consts.tile([P, 1], FP16)
        nc.scalar.activation(
            out=warm_out, in_=warm_in,
            func=mybir.ActivationFunctionType.Exp, scale=1.0,
        )
        ident = consts.tile([P, P], F32)
        make_identity(nc, ident)

        # Per-job live state: set by emit_load / first emit_mm2.
        st = {}

        def emit_load(k):
            s = k % NSLOTS
            kc = kcs[s]
            # kqt columns: 0:P = k-tile 0, P:P+QW = qt, P+QW+t*P = k-tile
            # t (t>=1); host packs [k0 | qt | k1..] so the startup-critical
            # pieces stream in need-order.
            kqt = tp.tile([P, QW + S], qk_dt, tag="kqt", name=f"kqt{k}")
            vaug = vp.tile([P, NT, D + 2], FP16, tag="vaug", name=f"vaug{k}")
            if k == 0:
                # job 0 is latency-critical: k-tile 0 + first q half,
                # then the first V rows (exp bias source), then the rest
                # in arrival-order chunks
                nc.sync.dma_start(
                    out=kqt[:, 0 : P + 512], in_=kt_d[s][:, 0 : P + 512]
                )
                nc.sync.dma_start(
                    out=kqt[:, P + 512 : P + QW],
                    in_=kt_d[s][:, P + 512 : P + QW],
                )
                mid = min(5, kc)
                vmid = min(2, kc)
                nc.sync.dma_start(
                    out=vaug[:, 0:vmid, :], in_=v_d[s][:, 0:vmid, :]
                )
                if kc > 1:
                    nc.sync.dma_start(
                        out=kqt[:, P + QW : QW + mid * P],
                        in_=kt_d[s][:, P + QW : QW + mid * P],
                    )
                if kc > mid:
                    nc.sync.dma_start(
                        out=kqt[:, QW + mid * P : QW + kc * P],
                        in_=kt_d[s][:, QW + mid * P : QW + kc * P],
                    )
                if kc > vmid:
                    nc.sync.dma_start(
                        out=vaug[:, vmid:kc, :], in_=v_d[s][:, vmid:kc, :]
                    )
            else:
                nc.sync.dma_start(
                    out=kqt[:, 0 : QW + kc * P],
                    in_=kt_d[s][:, 0 : QW + kc * P],
                )
                nc.sync.dma_start(
                    out=vaug[:, 0:kc, :], in_=v_d[s][:, 0:kc, :]
                )
            st[k] = dict(kc=kc, kqt=kqt, vaug=vaug, oaug=None, attn={})

        def emit_mm1(k, t):
            z = st[k]
            psc = ps.tile([P, QW], F32, tag="pmm", name="psc")
            z.setdefault("psc", {})[t] = psc
            kqt = z["kqt"]
            k_sl = (
                slice(0, P) if t == 0
                else slice(QW + t * P, QW + (t + 1) * P)
            )
            for j in range(2):
                nc.tensor.matmul(
                    psc[:, j * 512 : (j + 1) * 512],
                    kqt[:, k_sl],
                    kqt[:, P + j * 512 : P + (j + 1) * 512],
                    start=True, stop=True,
                )

        def emit_exp(k, t):
            z = st[k]
            psc = z["psc"].pop(t)
            attnT = atp.tile([P, QW], FP16, tag="attnT", name=f"at{k}_{t}")
            z["attn"][t] = attnT
            nc.scalar.activation(
                out=attnT, in_=psc,
                func=mybir.ActivationFunctionType.Exp,
                bias=z["vaug"][:, t, D + 1 : D + 2], scale=SCALE,
            )

        def emit_mm2(k, t):
            z = st[k]
            kc, vaug = z["kc"], z["vaug"]
            if z["oaug"] is None:
                # The final job borrows the scores (pmm) ring for its
                # accumulator + tro: the exp stream is over, and this
                # breaks the escape(j-1) -> first mm2(j) ring dependency
                # right where no exp work is left to hide it.
                tag = "pmm" if k == njobs_total - 1 else "oaug"
                z["oaug"] = ps.tile([D + 1, QW], F32, tag=tag,
                                    name=f"oaug{k}")
            oaug = z["oaug"]
            attnT = z["attn"].pop(t)
            noacc = abl == "noacc"  # timing probe: break the accumulation
            for j in range(2):
                q_sl = slice(j * 512, (j + 1) * 512)
                nc.tensor.matmul(
                    oaug[:, q_sl], vaug[:, t, 0 : D + 1], attnT[:, q_sl],
                    start=(t == 0) or noacc, stop=(t == kc - 1) or noacc,
                )

        def emit_tail_a(k, on_act=False):
            # PSUM escape as soon as the accumulation closes. For the very
            # last job the exp stream is over, so the then-idle ACT engine
            # takes the copy instead of the tail-congested DVE.
            z = st[k]
            oaug_sb = op_.tile([D + 1, QW], F32, tag="oaugsb", name=f"ob{k}")
            if on_act:
                nc.scalar.copy(oaug_sb, z["oaug"])
            else:
                nc.vector.tensor_copy(oaug_sb, z["oaug"])
            z["oaug_sb"] = oaug_sb

        def emit_tail_b(k, on_act=False):
            s = k % NSLOTS
            z = st.pop(k)
            oaug_sb = z["oaug_sb"]
            recip = sm.tile([P, NQB], F32, tag="recip", name=f"rc{k}")
            # padded tro tile (512B stride -> no transpose output crosses
            # a PSUM bank); aliases the oaug PSUM ring (tag-shared), or
            # the dead scores ring for the final job.
            tro = ps.tile([P, NQB, P], F32,
                          tag="pmm" if on_act else "oaug", name="tro")
            for qi in range(NQB):
                nc.tensor.transpose(
                    tro[:, qi, 0 : D + 1],
                    oaug_sb[:, qi * P : (qi + 1) * P],
                    ident[0 : D + 1, 0 : D + 1],
                )
            nc.vector.reciprocal(recip, tro[:, :, D : D + 1])
            if on_act:
                # Final jobs: ACT (idle, exp stream over) takes q-blocks
                # 0-3, DVE takes 4-7 — separate tiles, contiguous halves,
                # one sync boundary, natural output layout.
                out_lo = op_.tile([P, NQB // 2, D], F32, tag="outsb",
                                  name=f"osl{k}")
                out_hi = op_.tile([P, NQB // 2, D], F32, tag="outsb2",
                                  name=f"osh{k}")
                # DVE half emitted FIRST: otherwise the framework expresses
                # the DVE muls' transpose dependency transitively through
                # the ACT muls' completion counter, serializing the engines.
                for qi in range(NQB // 2, NQB):
                    nc.vector.tensor_scalar_mul(
                        out_hi[:, qi - NQB // 2, :], tro[:, qi, 0:D],
                        recip[:, qi : qi + 1],
                    )
                for qi in range(NQB // 2):
                    nc.scalar.mul(
                        out_lo[:, qi, :], tro[:, qi, 0:D],
                        recip[:, qi : qi + 1],
                    )
                # spread the final out-DMAs over two DGE queues
                nc.gpsimd.dma_start(
                    out=o_d[s][:, NQB // 2 :, :], in_=out_hi
                )
                nc.sync.dma_start(
                    out=o_d[s][:, 0 : NQB // 2, :], in_=out_lo
                )
            else:
                out_sb = op_.tile([P, NQB, D], F32, tag="outsb",
                                  name=f"os{k}")
                for qi in range(NQB):
                    nc.vector.tensor_scalar_mul(
                        out_sb[:, qi, :], tro[:, qi, 0:D],
                        recip[:, qi : qi + 1],
                    )
                nc.sync.dma_start(out=o_d[s], in_=out_sb)

        if loop > 1:
            ctx.enter_context(tc.For_i(0, loop))
        # Software pipeline across (rep, job, k-tile) steps: mm2 trails
        # mm1+exp by LAG steps; tail part a (escape) at the job's last
        # mm2, part b right after the NEXT job's first mm2 (so the tro
        # allocation lands in the PSUM ring slot just vacated by this
        # job's own oaug).
        # Pair-granularity pipeline: each step handles TWO k-tiles
        # (mm1 x2 then exp x2; later mm2 x2 batched) so the PE stream
        # runs in long same-kind blocks — halves the per-instruction
        # semaphore-wait and weight-switch overhead that measures ~0.6us
        # per k-tile on real hardware when interleaved singly.
        LAG = 2  # pairs
        steps = []
        for r in range(repeat):
            for s in range(NSLOTS):
                kc = kcs[s]
                for t in range(0, kc, 2):
                    ts = (t, t + 1) if t + 1 < kc else (t,)
                    steps.append((r * NSLOTS + s, ts, t + 2 >= kc))
        pending = []      # (job, (tiles,), is_last) with mm2 not yet emitted
        tailb_todo = []   # jobs whose tail_b waits for the next oaug alloc

        def drain_one():
            k_, ts_, last_ = pending.pop(0)
            if abl == "nomm2":
                if last_:
                    st.pop(k_)
                return
            for t_ in ts_:
                emit_mm2(k_, t_)
            if abl == "notail":
                if last_:
                    st.pop(k_)
                return
            if last_:
                emit_tail_a(k_, on_act=(k_ == njobs_total - 1))
                emit_tail_b(k_, on_act=(k_ >= njobs_total - ACT_TAIL_JOBS))

        njobs_total = repeat * NSLOTS
        emit_load(0)
        for k, ts, is_last in steps:
            if k + 1 < njobs_total and ts[0] == min(2, (kcs[k % NSLOTS] - 1) & ~1):
                emit_load(k + 1)
            for t in ts:
                emit_mm1(k, t)
            for t in ts:
                emit_exp(k, t)
            pending.append((k, ts, is_last))
            if len(pending) > LAG:
                drain_one()
        while pending:
            drain_one()

    nc.compile()
    return nc


def _plan(valid_lens):
    # 64 jobs = (batch, q-half); sort by valid_len desc, group 8 per
    # slot; every core gets one job per slot; slot trip count = ceil of
    # the group max / P.
    vl = np.asarray(valid_lens).astype(np.int64)
    jobs_vl = np.repeat(vl, NHALF)             # job j = (batch j//2, half j%2)
    order = np.argsort(-jobs_vl, kind="stable")
    assign = order.reshape(NSLOTS, NCORES)     # job indices
    kcs = []
    for s_ in range(NSLOTS):
        m = int(jobs_vl[assign[s_]].max())
        kcs.append(max(1, -(-m // P)))
    return assign, kcs


def make_in_maps(queries, keys, values, vl, assign):
    key_ids = np.arange(S, dtype=np.int64)
    qk_np = np.float16 if MM1_FP16 else np.float32
    qT = np.ascontiguousarray(queries.transpose(0, 2, 1).astype(qk_np))
    kT = np.ascontiguousarray(keys.transpose(0, 2, 1).astype(qk_np))
    B = len(vl)
    # V packed as [V | 1 | mask]: one contiguous ~2KB DMA descriptor per
    # partition per job (see the v_d comment in _build_program).
    vpack = np.empty((B, P, NT, D + 2), dtype=np.float16)
    vpack[:, :, :, 0:D] = values.reshape(B, NT, P, D).transpose(0, 2, 1, 3)
    vpack[:, :, :, D] = 1.0
    masks = np.where(
        key_ids[None, :] < np.asarray(vl)[:, None], 0.0, MASK_FP16
    ).reshape(B, NT, P).transpose(0, 2, 1)
    vpack[:, :, :, D + 1] = masks
    in_maps = []
    for c in range(NCORES):
        jobs = assign[:, c]
        b_idx = jobs // NHALF
        h_idx = jobs % NHALF
        # kt input = [k-tile0 | qt | k-tiles 1..] packed per job; rows
        # 64-127 zeroed (uniform 128-row PE contraction, see _build_program)
        kq = np.zeros((NSLOTS, P, QW + S), dtype=qk_np)
        for s_, (b, h) in enumerate(zip(b_idx, h_idx)):
            kq[s_, 0:D, 0:P] = kT[b, :, 0:P]
            kq[s_, 0:D, P : P + QW] = qT[b, :, h * QW : (h + 1) * QW]
            kq[s_, 0:D, P + QW :] = kT[b, :, P:]
        in_maps.append(
            {
                "kt": kq,
                "v": np.ascontiguousarray(vpack[b_idx]),
            }
        )
    return in_maps


def kernel(queries, keys, values, valid_lens):
    global LAST_RESULT
    queries = np.ascontiguousarray(np.asarray(queries), dtype=np.float32)
    keys = np.ascontiguousarray(np.asarray(keys), dtype=np.float32)
    values = np.ascontiguousarray(np.asarray(values), dtype=np.float32)
    vl = np.asarray(valid_lens).astype(np.int64)
    B = queries.shape[0]
    assert queries.shape == (B, S, D) and B * NHALF == NCORES * NSLOTS

    assign, kcs = _plan(vl)
    key = tuple(kcs)
    nc = _PROGRAM_CACHE.get(key)
    if nc is None:
        nc = _PROGRAM_CACHE[key] = _build_program(kcs)
    in_maps = make_in_maps(queries, keys, values, vl, assign)

    import os
    try:
        LAST_RESULT = run_bass_kernel_spmd(
            nc, in_maps, core_ids=list(range(NCORES))
        )
    except ModuleNotFoundError:
        os.environ["BASS_NEVER_TRACE"] = "1"
        LAST_RESULT = run_bass_kernel_spmd(
            nc, in_maps, core_ids=list(range(NCORES))
        )

    out = np.empty((B, S, D), dtype=np.float32)
    for c in range(NCORES):
        o = LAST_RESULT.results[c]["out"]  # [NSLOTS, P, NQB, D]
        for s_ in range(NSLOTS):
            j = assign[s_, c]
            b, h = j // NHALF, j % NHALF
            out[b, h * QW : (h + 1) * QW] = (
                o[s_].transpose(1, 0, 2).reshape(QW, D)
            )
    return out


# revision 64
# speedup vs baseline: 1.0453x; 1.0453x over previous
# Dot-product attention with per-batch key masking (d2l masked_softmax
# style), distributed over 8 NeuronCores:
#
#   out[b] = softmax(mask(Q[b] @ K[b]^T / sqrt(D), valid_lens[b])) @ V[b]
#
# Shapes: Q/K/V [32, 2048, 64] f32, valid_lens [32] i32.
#
# Strategy (HW-measured 89.7us vs 124.6us for the v1 whole-batch design):
# per core 8 "jobs", each = (batch, query-half) with q-width 1024. The 64
# (batch, half) jobs are sorted by valid_len and grouped 8-per-slot so
# slot s of every core shares a compile-time k-tile trip count kcs[s] —
# half-batch grouping cuts the sum of per-slot maxima 41 -> 38 full-ktile
# equivalents of ACT work (the bottleneck engine).
#
# Per (job, k-tile): mm1 scoresT[128k, 1024q] = K_tile.T @ QT (fp16,
# d-major, host-packed [k-tile0 | qt | k-tiles 1..] with rows 64-127
# zero-padded so EVERY matmul contracts over 128 partitions — mixed
# 64/128-row configs cost ~110ns per switch on HW); ONE [128,1024] exp
# on ACT (bias = per-key mask from the packed V tensor, so masked keys
# become exactly 0); mm2 accumulates oaug[65, 1024] = [V|1].T @ attnT
# in PSUM (row 64 = softmax denominator).
#
# HW-governing choices (cost model alone misses all of these):
# - PAIR-GRANULARITY pipeline: two k-tiles per step (mm1 x4, exp x2,
#   mm2 x4 in same-kind blocks). Interleaving mm1/mm2 singly costs
#   ~0.6us/k-tile in PE sem-wait/weight-switch overhead on HW (152us!).
# - V packed host-side as [V | 1 | mask] rows: the whole per-job V block
#   loads as ONE ~2KB contiguous descriptor per partition (the naive
#   strided V load = 128*kc 128-byte descriptors, far past the DMA
#   roofline on HW).
# - PSUM: scores ring 2 x [128,1024] + oaug ring 2 x [65,1024] = all 8
#   banks; the tail's tro tile aliases the oaug ring; the final job
#   borrows the (then dead) scores ring instead, removing the
#   escape->mm2 ring dependency right where nothing hides it.
# - Tail per job: DVE PSUM escape, 8 PE transposes back to [q, d'],
#   reciprocal of the denominator column, 8 per-q-block scalar muls,
#   one output DMA. tail_b is emitted at the NEXT job's first mm2
#   drain; the final jobs split muls DVE/ACT (ACT idle then; DVE half
#   emitted first or the framework serializes the engines transitively).
# - Dummy exp at t=0 so the one-time ACT table load overlaps the DMAs;
#   job-0 loads stream in need-order ([k0|q-half], V rows, rest).

import numpy as np
import ml_dtypes
from contextlib import ExitStack

import concourse.bass as bass
import concourse.bacc as bacc
import concourse.mybir as mybir
import concourse.tile as tile
from concourse.masks import make_identity
from concourse.bass_utils import run_bass_kernel_spmd

P = 128
S = 2048
D = 64
NT = S // P
NCORES = 8
NSLOTS = 8          # jobs per core
QW = 1024           # query width per job
NQB = QW // P       # q-blocks per job (8)
NHALF = S // QW     # query halves per batch (2)
MASK_NEG = -1.0e6
MASK_FP16 = -60000.0  # mask bias packed in fp16; exp underflows to 0 either way
SCALE = 0.125
F32 = mybir.dt.float32
F32R = mybir.dt.float32r
FP16 = mybir.dt.float16
BF16 = mybir.dt.bfloat16

MM1_FP16 = True
# The final jobs' tail multiplies split between DVE and the (then idle)
# ACT engine — contiguous halves so only one cross-engine sync boundary
# exists per job.
ACT_TAIL_JOBS = 2

LAST_RESULT = None
_PROGRAM_CACHE = {}


def _build_program(kcs, repeat=1, loop=1):
    nc = bacc.Bacc("TRN2", target_bir_lowering=False, debug=False)

    qk_dt = FP16 if MM1_FP16 else F32R
    # qt and K packed in one DRAM tensor ([k-tile0 | qt | k-tiles 1..]) so
    # the latency-critical first load (k-tile 0 + qt) is a single DMA.
    # Rows 64-127 are host-zeroed so every matmul (mm1 AND mm2) contracts
    # over the full 128 partitions: uniform PE row configuration avoids
    # the per-switch array reconfiguration cost measured on hardware.
    kt_d = nc.dram_tensor(
        "kt", [NSLOTS, P, QW + S], qk_dt, kind="ExternalInput"
    )
    # V packed host-side as [V | 1 | mask] per k-tile row: the whole
    # (V, softmax-denominator ones column, exp mask bias) block loads as
    # ONE contiguous ~2KB descriptor per partition — the naive strided
    # [P, 0:kc, D] V load costs 128*kc tiny 128B descriptors, which on
    # real hardware is expensive enough to push DMA past the ACT roofline.
    v_d = nc.dram_tensor(
        "v", [NSLOTS, P, NT, D + 2], FP16, kind="ExternalInput"
    )
    o_d = nc.dram_tensor("out", [NSLOTS, P, NQB, D], F32, kind="ExternalOutput")

    with ExitStack() as ctx:
        tc = ctx.enter_context(tile.TileContext(nc))
        consts = ctx.enter_context(tc.tile_pool(name="consts", bufs=1))
        tp = ctx.enter_context(tc.tile_pool(name="tp", bufs=2))
        vp = ctx.enter_context(tc.tile_pool(name="vp", bufs=2))
        atp = ctx.enter_context(tc.tile_pool(name="atp", bufs=6))
        op_ = ctx.enter_context(tc.tile_pool(name="op_", bufs=2))
        sm = ctx.enter_context(tc.tile_pool(name="sm", bufs=2))
        ps = ctx.enter_context(tc.tile_pool(name="ps", bufs=2, space="PSUM"))

        # Dummy exp at t=0: triggers the one-time ACT table load with no
        # data deps so it overlaps the initial DMAs (DVE memset so the
        # dependency clears within ~0.1us).
        warm_in = consts.tile([P, 1], F32)
        nc.vector.memset(warm_in, 0.0)
        warm_out = consts.tile([P, 1], FP16)
        nc.scalar.activation(
            out=warm_out, in_=warm_in,
            func=mybir.ActivationFunctionType.Exp, scale=1.0,
        )
        ident = consts.tile([P, P], F32)
        make_identity(nc, ident)

        # Per-job live state: set by emit_load / first emit_mm2.
        st = {}

        def emit_load(k):
            s = k % NSLOTS
            kc = kcs[s]
            # kqt columns: 0:P = k-tile 0, P:P+QW = qt, P+QW+t*P = k-tile
            # t (t>=1); host packs [k0 | qt | k1..] so the startup-critical
            # pieces stream in need-order.
            kqt = tp.tile([P, QW + S], qk_dt, tag="kqt", name=f"kqt{k}")
            vaug = vp.tile([P, NT, D + 2], FP16, tag="vaug", name=f"vaug{k}")
            if k == 0:
                # job 0 is latency-critical: k-tile 0 + first q half,
                # then the first V rows (exp bias source), then the rest
                # in arrival-order chunks
                nc.sync.dma_start(
                    out=kqt[:, 0 : P + 512], in_=kt_d[s][:, 0 : P + 512]
                )
                nc.sync.dma_start(
                    out=kqt[:, P + 512 : P + QW],
                    in_=kt_d[s][:, P + 512 : P + QW],
                )
                mid = min(5, kc)
                vmid = min(2, kc)
                nc.sync.dma_start(
                    out=vaug[:, 0:vmid, :], in_=v_d[s][:, 0:vmid, :]
                )
                if kc > 1:
                    nc.sync.dma_start(
                        out=kqt[:, P + QW : QW + mid * P],
                        in_=kt_d[s][:, P + QW : QW + mid * P],
                    )
                if kc > mid:
                    nc.sync.dma_start(
                        out=kqt[:, QW + mid * P : QW + kc * P],
                        in_=kt_d[s][:, QW + mid * P : QW + kc * P],
                    )
                if kc > vmid:
                    nc.sync.dma_start(
                        out=vaug[:, vmid:kc, :], in_=v_d[s][:, vmid:kc, :]
                    )
            else:
                nc.sync.dma_start(
                    out=kqt[:, 0 : QW + kc * P],
                    in_=kt_d[s][:, 0 : QW + kc * P],
                )
                nc.sync.dma_start(
                    out=vaug[:, 0:kc, :], in_=v_d[s][:, 0:kc, :]
                )
            st[k] = dict(kc=kc, kqt=kqt, vaug=vaug, oaug=None, attn={})

        def emit_mm1(k, t):
            z = st[k]
            psc = ps.tile([P, QW], F32, tag="pmm", name="psc")
            z.setdefault("psc", {})[t] = psc
            kqt = z["kqt"]
            k_sl = (
                slice(0, P) if t == 0
                else slice(QW + t * P, QW + (t + 1) * P)
            )
            for j in range(2):
                nc.tensor.matmul(
                    psc[:, j * 512 : (j + 1) * 512],
                    kqt[:, k_sl],
                    kqt[:, P + j * 512 : P + (j + 1) * 512],
                    start=True, stop=True,
                )

        def emit_exp(k, t):
            z = st[k]
            psc = z["psc"].pop(t)
            attnT = atp.tile([P, QW], FP16, tag="attnT", name=f"at{k}_{t}")
            z["attn"][t] = attnT
            nc.scalar.activation(
                out=attnT, in_=psc,
                func=mybir.ActivationFunctionType.Exp,
                bias=z["vaug"][:, t, D + 1 : D + 2], scale=SCALE,
            )

        def emit_mm2(k, t):
            z = st[k]
            kc, vaug = z["kc"], z["vaug"]
            if z["oaug"] is None:
                # The final job borrows the scores (pmm) ring for its
                # accumulator + tro: the exp stream is over, and this
                # breaks the escape(j-1) -> first mm2(j) ring dependency
                # right where no exp work is left to hide it.
                tag = "pmm" if k == njobs_total - 1 else "oaug"
                z["oaug"] = ps.tile([D + 1, QW], F32, tag=tag,
                                    name=f"oaug{k}")
            oaug = z["oaug"]
            attnT = z["attn"].pop(t)
            for j in range(2):
                q_sl = slice(j * 512, (j + 1) * 512)
                nc.tensor.matmul(
                    oaug[:, q_sl], vaug[:, t, 0 : D + 1], attnT[:, q_sl],
                    start=(t == 0), stop=(t == kc - 1),
                )

        def emit_tail_a(k, on_act=False):
            # PSUM escape as soon as the accumulation closes. For the very
            # last job the exp stream is over, so the then-idle ACT engine
            # takes the copy instead of the tail-congested DVE.
            z = st[k]
            oaug_sb = op_.tile([D + 1, QW], F32, tag="oaugsb", name=f"ob{k}")
            if on_act:
                nc.scalar.copy(oaug_sb, z["oaug"])
            else:
                nc.vector.tensor_copy(oaug_sb, z["oaug"])
            z["oaug_sb"] = oaug_sb

        def emit_tail_b(k, on_act=False):
            s = k % NSLOTS
            z = st.pop(k)
            oaug_sb = z["oaug_sb"]
            recip = sm.tile([P, NQB], F32, tag="recip", name=f"rc{k}")
            # padded tro tile (512B stride -> no transpose output crosses
            # a PSUM bank); aliases the oaug PSUM ring (tag-shared), or
            # the dead scores ring for the final job.
            tro = ps.tile([P, NQB, P], F32,
                          tag="pmm" if on_act else "oaug", name="tro")
            for qi in range(NQB):
                nc.tensor.transpose(
                    tro[:, qi, 0 : D + 1],
                    oaug_sb[:, qi * P : (qi + 1) * P],
                    ident[0 : D + 1, 0 : D + 1],
                )
            nc.vector.reciprocal(recip, tro[:, :, D : D + 1])
            if on_act:
                # Final jobs: ACT (idle, exp stream over) takes q-blocks
                # 0-3, DVE takes 4-7 — separate tiles, contiguous halves,
                # one sync boundary, natural output layout.
                out_lo = op_.tile([P, NQB // 2, D], F32, tag="outsb",
                                  name=f"osl{k}")
                out_hi = op_.tile([P, NQB // 2, D], F32, tag="outsb2",
                                  name=f"osh{k}")
                # DVE half emitted FIRST: otherwise the framework expresses
                # the DVE muls' transpose dependency transitively through
                # the ACT muls' completion counter, serializing the engines.
                for qi in range(NQB // 2, NQB):
                    nc.vector.tensor_scalar_mul(
                        out_hi[:, qi - NQB // 2, :], tro[:, qi, 0:D],
                        recip[:, qi : qi + 1],
                    )
                for qi in range(NQB // 2):
                    nc.scalar.mul(
                        out_lo[:, qi, :], tro[:, qi, 0:D],
                        recip[:, qi : qi + 1],
                    )
                # spread the final out-DMAs over two DGE queues
                nc.gpsimd.dma_start(
                    out=o_d[s][:, NQB // 2 :, :], in_=out_hi
                )
                nc.sync.dma_start(
                    out=o_d[s][:, 0 : NQB // 2, :], in_=out_lo
                )
            else:
                out_sb = op_.tile([P, NQB, D], F32, tag="outsb",
                                  name=f"os{k}")
                for qi in range(NQB):
                    nc.vector.tensor_scalar_mul(
                        out_sb[:, qi, :], tro[:, qi, 0:D],
                        recip[:, qi : qi + 1],
                    )
                nc.sync.dma_start(out=o_d[s], in_=out_sb)

        if loop > 1:
            ctx.enter_context(tc.For_i(0, loop))
        # Software pipeline across (rep, job, k-tile) steps: mm2 trails
        # mm1+exp by LAG steps; tail part a (escape) at the job's last
        # mm2, part b right after the NEXT job's first mm2 (so the tro
        # allocation lands in the PSUM ring slot just vacated by this
        # job's own oaug).
        # Pair-granularity pipeline: each step handles TWO k-tiles
        # (mm1 x2 then exp x2; later mm2 x2 batched) so the PE stream
        # runs in long same-kind blocks — halves the per-instruction
        # semaphore-wait and weight-switch overhead that measures ~0.6us
        # per k-tile on real hardware when interleaved singly.
        LAG = 2  # pairs
        steps = []
        for r in range(repeat):
            for s in range(NSLOTS):
                kc = kcs[s]
                for t in range(0, kc, 2):
                    ts = (t, t + 1) if t + 1 < kc else (t,)
                    steps.append((r * NSLOTS + s, ts, t + 2 >= kc))
        pending = []      # (job, (tiles,), is_last) with mm2 not yet emitted
        tailb_todo = []   # jobs whose tail_b waits for the next oaug alloc

        def drain_one():
            k_, ts_, last_ = pending.pop(0)
            first_ = ts_[0] == 0
            for t_ in ts_:
                emit_mm2(k_, t_)
            # defer the second-to-last job's tail_b past the last job's
            # escape so the escape isn't queued behind tail multiplies
            if first_ and tailb_todo and k_ != njobs_total - 1:
                kb = tailb_todo.pop(0)
                emit_tail_b(kb, on_act=(kb >= njobs_total - ACT_TAIL_JOBS))
            if last_:
                emit_tail_a(k_, on_act=(k_ == njobs_total - 1))
                tailb_todo.append(k_)

        njobs_total = repeat * NSLOTS
        emit_load(0)
        for k, ts, is_last in steps:
            if k + 1 < njobs_total and ts[0] == min(2, (kcs[k % NSLOTS] - 1) & ~1):
                emit_load(k + 1)
            for t in ts:
                emit_mm1(k, t)
            for t in ts:
                emit_exp(k, t)
            pending.append((k, ts, is_last))
            if len(pending) > LAG:
                drain_one()
        while pending:
            drain_one()
        while tailb_todo:
            k_ = tailb_todo.pop(0)
            emit_tail_b(k_, on_act=(k_ >= njobs_total - ACT_TAIL_JOBS))

    nc.compile()
    return nc


def _plan(valid_lens):
    # 64 jobs = (batch, q-half); sort by valid_len desc, group 8 per
    # slot; every core gets one job per slot; slot trip count = ceil of
    # the group max / P.
    vl = np.asarray(valid_lens).astype(np.int64)
    jobs_vl = np.repeat(vl, NHALF)             # job j = (batch j//2, half j%2)
    order = np.argsort(-jobs_vl, kind="stable")
    assign = order.reshape(NSLOTS, NCORES)     # job indices
    kcs = []
    for s_ in range(NSLOTS):
        m = int(jobs_vl[assign[s_]].max())
        kcs.append(max(1, -(-m // P)))
    return assign, kcs


def make_in_maps(queries, keys, values, vl, assign):
    key_ids = np.arange(S, dtype=np.int64)
    qk_np = np.float16 if MM1_FP16 else np.float32
    qT = np.ascontiguousarray(queries.transpose(0, 2, 1).astype(qk_np))
    kT = np.ascontiguousarray(keys.transpose(0, 2, 1).astype(qk_np))
    B = len(vl)
    # V packed as [V | 1 | mask]: one contiguous ~2KB DMA descriptor per
    # partition per job (see the v_d comment in _build_program).
    vpack = np.empty((B, P, NT, D + 2), dtype=np.float16)
    vpack[:, :, :, 0:D] = values.reshape(B, NT, P, D).transpose(0, 2, 1, 3)
    vpack[:, :, :, D] = 1.0
    masks = np.where(
        key_ids[None, :] < np.asarray(vl)[:, None], 0.0, MASK_FP16
    ).reshape(B, NT, P).transpose(0, 2, 1)
    vpack[:, :, :, D + 1] = masks
    in_maps = []
    for c in range(NCORES):
        jobs = assign[:, c]
        b_idx = jobs // NHALF
        h_idx = jobs % NHALF
        # kt input = [k-tile0 | qt | k-tiles 1..] packed per job; rows
        # 64-127 zeroed (uniform 128-row PE contraction, see _build_program)
        kq = np.zeros((NSLOTS, P, QW + S), dtype=qk_np)
        for s_, (b, h) in enumerate(zip(b_idx, h_idx)):
            kq[s_, 0:D, 0:P] = kT[b, :, 0:P]
            kq[s_, 0:D, P : P + QW] = qT[b, :, h * QW : (h + 1) * QW]
            kq[s_, 0:D, P + QW :] = kT[b, :, P:]
        in_maps.append(
            {
                "kt": kq,
                "v": np.ascontiguousarray(vpack[b_idx]),
            }
        )
    return in_maps


def kernel(queries, keys, values, valid_lens):
    global LAST_RESULT
    queries = np.ascontiguousarray(np.asarray(queries), dtype=np.float32)
    keys = np.ascontiguousarray(np.asarray(keys), dtype=np.float32)
    values = np.ascontiguousarray(np.asarray(values), dtype=np.float32)
    vl = np.asarray(valid_lens).astype(np.int64)
    B = queries.shape[0]
    assert queries.shape == (B, S, D) and B * NHALF == NCORES * NSLOTS

    assign, kcs = _plan(vl)
    key = tuple(kcs)
    nc = _PROGRAM_CACHE.get(key)
    if nc is None:
        nc = _PROGRAM_CACHE[key] = _build_program(kcs)
    in_maps = make_in_maps(queries, keys, values, vl, assign)

    import os
    try:
        LAST_RESULT = run_bass_kernel_spmd(
            nc, in_maps, core_ids=list(range(NCORES))
        )
    except ModuleNotFoundError:
        os.environ["BASS_NEVER_TRACE"] = "1"
        LAST_RESULT = run_bass_kernel_spmd(
            nc, in_maps, core_ids=list(range(NCORES))
        )

    out = np.empty((B, S, D), dtype=np.float32)
    for c in range(NCORES):
        o = LAST_RESULT.results[c]["out"]  # [NSLOTS, P, NQB, D]
        for s_ in range(NSLOTS):
            j = assign[s_, c]
            b, h = j // NHALF, j % NHALF
            out[b, h * QW : (h + 1) * QW] = (
                o[s_].transpose(1, 0, 2).reshape(QW, D)
            )
    return out
